# revision 10
# baseline (speedup 1.0000x reference)
"""DynamicLinear (MoE routing) Trainium2 Bass kernel.

Math (per sample b):
    out[b] = sum_k attn[b,k] * (x[b] @ W[k].T + bias[k])
           = sum_k attn[b,k] * (x[b] @ W[k].T) + attn[b] @ bias

Sharding: 8 cores in a 2x4 grid over (batch, out_features).
Each core computes out[b_half, o_quarter] from x[b_half] (16 MiB fp32),
W[:, o_quarter, :] (16 MiB fp32) -- no cross-core communication.

Per-core pipeline (expert-outer so TensorE starts once W[0] is staged):
  1. gpsimd casting DMAs: x / W fp32 -> bf16 DRAM staging.
  2. xbar DMA transposes (bf16, DRAM -> SBUF, one instruction per
     expert / x-group): wT[k] [128,16(ii),512(o)], xT[g] [128,16,512(b)].
     W transposes issue on the sync ring, x transposes on the scalar
     ring -- the ucode transpose blocks its issuing engine, so the two
     streams must not share one ring.
  3. TensorE, for k in 4: for b_tile in 16: accumulate 16 matmul passes
     (K=128 contraction, N=512 contiguous moving) into one PSUM bank.
  4. ACT+DVE combine into per-b_tile SBUF accumulators:
     acc[t] = sum_k attn[:,k]*(bias[k] + psum_k), attn as per-partition
     scalar (b lives on the partition dim).
  5. DMA acc -> out after the last expert.
"""

import numpy as np

_B, _K, _IN, _OUT = 4096, 4, 2048, 2048
_GRID_B, _GRID_O = 2, 4
_BL = _B // _GRID_B      # 2048 batch rows per core
_OL = _OUT // _GRID_O    # 512 out cols per core
_NBT = _BL // 128        # 16 b tiles
_NIT = _IN // 128        # 16 contraction tiles
_NOT = _OL // 128        # 4 o row-tiles of W
_XG = 512                # batch rows per x-transpose group
_NG = _BL // _XG         # 4 groups

_CACHE = {}
LAST_RESULTS = None


def _build_program():
    import concourse.bass as bass
    import concourse.tile as tile
    from concourse import bacc, mybir

    f32 = mybir.dt.float32
    bf16 = mybir.dt.bfloat16
    MULT = mybir.AluOpType.mult
    ADD = mybir.AluOpType.add
    COPY = mybir.ActivationFunctionType.Copy

    nc = bacc.Bacc("TRN2", target_bir_lowering=False, debug=False)
    x = nc.dram_tensor("x", [_BL, _IN], f32, kind="ExternalInput").ap()
    attn = nc.dram_tensor("attn", [_BL, _K], f32, kind="ExternalInput").ap()
    w = nc.dram_tensor("w", [_K, _OL, _IN], f32, kind="ExternalInput").ap()
    bias = nc.dram_tensor("bias", [_K, _OL], f32, kind="ExternalInput").ap()
    out = nc.dram_tensor("out", [_BL, _OL], f32, kind="ExternalOutput").ap()

    with tile.TileContext(nc) as tc:
        with (
            tc.tile_pool(name="dram", bufs=1, space="DRAM") as dram,
            tc.tile_pool(name="wT", bufs=2) as wTp,
            tc.tile_pool(name="xnat", bufs=6) as xnatp,
            tc.tile_pool(name="xT", bufs=_NBT) as xTp,
            tc.tile_pool(name="singles", bufs=1) as singles,
            tc.tile_pool(name="acc", bufs=_NBT) as accp,
            tc.tile_pool(name="psum", bufs=8, space="PSUM") as psump,
        ):
            wbf = dram.tile([_K, _OL, _IN], bf16)

            def cast_w(k):
                for oi in range(_NOT):
                    nc.gpsimd.dma_start(
                        out=wbf[k, oi * 128:(oi + 1) * 128, :],
                        in_=w[k, oi * 128:(oi + 1) * 128, :],
                    )

            def trans_w(k):
                # wT[k][i_in, ii, o] = W[k][o, ii*128 + i_in]  (sync ring)
                wt = wTp.tile([128, _NIT, _OL], bf16, tag="wT", name=f"wT{k}")
                nc.sync.dma_start_transpose(wt, wbf[k])
                return wt

            def stage_x(t):
                # cast straight into SBUF, then SBUF->SBUF xbar transpose
                # on the scalar ring: xT[t][i_in, ii, b] = x[t*128+b, i]
                xn = xnatp.tile([128, _IN], bf16, tag="xnat", name=f"xnat{t}")
                nc.gpsimd.dma_start(out=xn, in_=x[t * 128:(t + 1) * 128, :])
                xt = xTp.tile([128, _NIT, 128], bf16, tag="xT", name=f"xT{t}")
                nc.scalar.dma_start_transpose(xt, xn)
                return xt

            # attn for all b_tiles in one strided load, b on partitions:
            # attn_sb[p, t, k] = attn[t*128 + p, k]   (sync ring)
            attn_sb = singles.tile([128, _NBT, _K], f32)
            attn_src = bass.AP(
                tensor=attn.tensor,
                offset=attn.offset,
                ap=[[_K, 128], [128 * _K, _NBT], [1, _K]],
            )
            nc.sync.dma_start(out=attn_sb, in_=attn_src)

            # staging in need-order; the two transpose rings drain FIFO
            cast_w(0)
            xT = [stage_x(0), stage_x(1), stage_x(2), stage_x(3)]
            wT = {0: trans_w(0)}
            bias_rep = singles.tile([128, _K, _OL], f32)
            nc.gpsimd.dma_start(
                out=bias_rep,
                in_=bass.AP(
                    tensor=bias.tensor,
                    offset=bias.offset,
                    ap=[[0, 128], bias.ap[0], bias.ap[1]],
                ),
            )
            for t in range(4, 8):
                xT.append(stage_x(t))
            cast_w(1)
            wT[1] = trans_w(1)
            for t in range(8, _NBT):
                xT.append(stage_x(t))
            cast_w(2)
            wT[2] = trans_w(2)
            cast_w(3)
            wT[3] = trans_w(3)

            acc = [None] * _NBT
            for k in range(_K):
                for t in range(_NBT):
                    ps = psump.tile([128, _OL], f32, tag="ps",
                                    name=f"ps{k}_{t}")
                    for ii in range(_NIT):
                        nc.tensor.matmul(
                            ps,
                            lhsT=xT[t][:, ii, :],
                            rhs=wT[k][:, ii, :],
                            start=(ii == 0), stop=(ii == _NIT - 1),
                        )
                    a_sc = attn_sb[:, t, :]
                    if k == 0:
                        # init acc with the full bias combination (DVE only;
                        # the scalar ring is reserved for x transposes)
                        at = accp.tile([128, _OL], f32, tag="acc",
                                       name=f"acc{t}")
                        acc[t] = at
                        nc.vector.tensor_scalar(
                            out=at, in0=bias_rep[:, 0, :],
                            scalar1=a_sc[:, 0:1], scalar2=None, op0=MULT,
                        )
                        for kk in range(1, _K):
                            nc.vector.scalar_tensor_tensor(
                                out=at, in0=bias_rep[:, kk, :],
                                scalar=a_sc[:, kk:kk + 1], in1=at,
                                op0=MULT, op1=ADD,
                            )
                    nc.vector.scalar_tensor_tensor(
                        out=acc[t], in0=ps, scalar=a_sc[:, k:k + 1],
                        in1=acc[t], op0=MULT, op1=ADD,
                    )
                    if k == _K - 1:
                        nc.sync.dma_start(
                            out=out[t * 128:(t + 1) * 128, :], in_=acc[t]
                        )

    nc.compile()
    return nc


def _get_program():
    if "nc" not in _CACHE:
        _CACHE["nc"] = _build_program()
    return _CACHE["nc"]


def _ensure_axon_hooks_importable():
    """bass_utils' trace branch imports antenv.axon_hooks, which the
    trimmed agent image may lack; stub it (hook=None) so a stray
    BASS_TRACE=1 degrades to an untraced run instead of crashing."""
    import sys
    import types

    try:
        import antenv.axon_hooks  # noqa: F401
        return
    except ImportError:
        pass
    mod = types.ModuleType("antenv.axon_hooks")
    mod._hook = None
    mod.get_axon_ntff_profile_hook = lambda: mod._hook

    def _set(h):
        mod._hook = h

    mod.set_axon_ntff_profile_hook = _set
    sys.modules["antenv.axon_hooks"] = mod
    try:
        import antenv
        antenv.axon_hooks = mod
    except ImportError:
        pass


def kernel(**inputs):
    global LAST_RESULTS
    from concourse.bass_utils import run_bass_kernel_spmd

    _ensure_axon_hooks_importable()

    x = np.ascontiguousarray(inputs["x"], dtype=np.float32)
    attn = np.ascontiguousarray(inputs["softmax_attention"], dtype=np.float32)
    w = np.ascontiguousarray(inputs["weight"], dtype=np.float32)
    b = np.ascontiguousarray(inputs["bias"], dtype=np.float32)

    nc = _get_program()
    in_maps = []
    for c in range(8):
        gb, go = divmod(c, _GRID_O)
        in_maps.append({
            "x": np.ascontiguousarray(x[gb * _BL:(gb + 1) * _BL]),
            "attn": np.ascontiguousarray(attn[gb * _BL:(gb + 1) * _BL]),
            "w": np.ascontiguousarray(w[:, go * _OL:(go + 1) * _OL, :]),
            "bias": np.ascontiguousarray(b[:, go * _OL:(go + 1) * _OL]),
        })

    res = run_bass_kernel_spmd(nc, in_maps, list(range(8)))
    LAST_RESULTS = res

    full = np.empty((_B, _OUT), dtype=np.float32)
    for c in range(8):
        gb, go = divmod(c, _GRID_O)
        full[gb * _BL:(gb + 1) * _BL, go * _OL:(go + 1) * _OL] = \
            res.results[c]["out"]
    return full


# revision 11
# speedup vs baseline: 1.3875x; 1.3875x over previous
"""DynamicLinear (MoE routing) Trainium2 Bass kernel.

Math (per sample b):
    out[b] = sum_k attn[b,k] * (x[b] @ W[k].T + bias[k])
           = sum_k attn[b,k] * (x[b] @ W[k].T) + attn[b] @ bias

Sharding: 8 cores in a 2x4 grid over (batch, out_features).
Each core computes out[b_half, o_quarter] from x[b_half] (16 MiB fp32)
and W[:, o_quarter, :] (16 MiB fp32) -- no cross-core communication.

The host ships x and W pre-transposed (i-major: xT [IN, BL] and
wT [K, IN, OL], still fp32) so the device needs no casts and no
transposes: every SBUF load is a plain strided HWDGE DMA that puts the
contraction dim on partitions, and the matmuls run as float32r (full
PE rate at N=512, fp32 storage, reduced-precision multiplies).

Per-core schedule (expert pairs keep SBUF small; x is streamed twice):
  phase 0: experts {0,1}  x  b_tiles 0..15
  phase 1: experts {2,3}  x  b_tiles 0..15
Per (b_tile, expert): 16 matmul passes (K=128 contraction, N=512
moving) accumulate in one PSUM bank; DVE combines
acc[t] = sum_k attn[:,k]*(bias[k] + psum_k) with attn as per-partition
scalar; out stores after the last expert.
"""

import numpy as np

_B, _K, _IN, _OUT = 4096, 4, 2048, 2048
_GRID_B, _GRID_O = 2, 4
_BL = _B // _GRID_B      # 2048 batch rows per core
_OL = _OUT // _GRID_O    # 512 out cols per core
_NBT = _BL // 128        # 16 b tiles
_NIT = _IN // 128        # 16 contraction tiles

_CACHE = {}
LAST_RESULTS = None


def _build_program():
    import concourse.bass as bass
    import concourse.tile as tile
    from concourse import bacc, mybir

    f32 = mybir.dt.float32
    f32r = mybir.dt.float32r
    MULT = mybir.AluOpType.mult
    ADD = mybir.AluOpType.add

    nc = bacc.Bacc("TRN2", target_bir_lowering=False, debug=False)
    xT = nc.dram_tensor("xT", [_IN, _BL], f32r, kind="ExternalInput").ap()
    attn = nc.dram_tensor("attn", [_BL, _K], f32, kind="ExternalInput").ap()
    wT = nc.dram_tensor("wT", [_K, _IN, _OL], f32r, kind="ExternalInput").ap()
    bias = nc.dram_tensor("bias", [_K, _OL], f32, kind="ExternalInput").ap()
    out = nc.dram_tensor("out", [_BL, _OL], f32, kind="ExternalOutput").ap()

    with tile.TileContext(nc) as tc:
        with (
            tc.tile_pool(name="wt", bufs=3) as wtp,
            tc.tile_pool(name="xt", bufs=5) as xtp,
            tc.tile_pool(name="singles", bufs=1) as singles,
            tc.tile_pool(name="acc", bufs=_NBT) as accp,
            tc.tile_pool(name="psum", bufs=8, space="PSUM") as psump,
        ):
            def load_w(k):
                # wt[k][i_in, ii, o] = wT[k, ii*128 + i_in, o]  (sync ring)
                t_ = wtp.tile([128, _NIT, _OL], f32r, tag="wt", name=f"wt{k}")
                src = bass.AP(
                    tensor=wT.tensor,
                    offset=wT.offset + k * _IN * _OL,
                    ap=[[_OL, 128], [128 * _OL, _NIT], [1, _OL]],
                )
                nc.sync.dma_start(out=t_, in_=src)
                return t_

            def load_x(t, phase):
                # xt[t][i_in, ii, b] = xT[ii*128 + i_in, t*128 + b]
                t_ = xtp.tile([128, _NIT, 128], f32r, tag="xt",
                              name=f"xt{phase}_{t}")
                src = bass.AP(
                    tensor=xT.tensor,
                    offset=xT.offset + t * 128,
                    ap=[[_BL, 128], [128 * _BL, _NIT], [1, 128]],
                )
                nc.scalar.dma_start(out=t_, in_=src)
                return t_

            # attn for all b_tiles, b on partitions:
            # attn_sb[p, t, k] = attn[t*128 + p, k]   (scalar ring)
            attn_sb = singles.tile([128, _NBT, _K], f32)
            attn_src = bass.AP(
                tensor=attn.tensor,
                offset=attn.offset,
                ap=[[_K, 128], [128 * _K, _NBT], [1, _K]],
            )
            nc.scalar.dma_start(out=attn_sb, in_=attn_src)

            # bias replicated across all 128 partitions (SWDGE, small)
            bias_rep = singles.tile([128, _K, _OL], f32)
            nc.gpsimd.dma_start(
                out=bias_rep,
                in_=bass.AP(
                    tensor=bias.tensor,
                    offset=bias.offset,
                    ap=[[0, 128], bias.ap[0], bias.ap[1]],
                ),
            )

            wt = {0: load_w(0), 1: load_w(1), 2: load_w(2)}
            acc = [None] * _NBT

            for phase in range(2):
                ks = (2 * phase, 2 * phase + 1)
                for t in range(_NBT):
                    if phase == 1 and t == 0:
                        wt[3] = load_w(3)
                    xt = load_x(t, phase)
                    a_sc = attn_sb[:, t, :]
                    for k in ks:
                        ps = psump.tile([128, _OL], f32, tag="ps",
                                        name=f"ps{k}_{t}")
                        for ii in range(_NIT):
                            nc.tensor.matmul(
                                ps,
                                lhsT=xt[:, ii, :],
                                rhs=wt[k][:, ii, :],
                                start=(ii == 0), stop=(ii == _NIT - 1),
                            )
                        if k == 0:
                            # init acc with the full bias combination (DVE)
                            at = accp.tile([128, _OL], f32, tag="acc",
                                           name=f"acc{t}")
                            acc[t] = at
                            nc.vector.tensor_scalar(
                                out=at, in0=bias_rep[:, 0, :],
                                scalar1=a_sc[:, 0:1], scalar2=None, op0=MULT,
                            )
                            for kk in range(1, _K):
                                nc.vector.scalar_tensor_tensor(
                                    out=at, in0=bias_rep[:, kk, :],
                                    scalar=a_sc[:, kk:kk + 1], in1=at,
                                    op0=MULT, op1=ADD,
                                )
                        nc.vector.scalar_tensor_tensor(
                            out=acc[t], in0=ps, scalar=a_sc[:, k:k + 1],
                            in1=acc[t], op0=MULT, op1=ADD,
                        )
                        if k == _K - 1:
                            nc.sync.dma_start(
                                out=out[t * 128:(t + 1) * 128, :],
                                in_=acc[t],
                            )

    nc.compile()
    return nc


def _get_program():
    if "nc" not in _CACHE:
        _CACHE["nc"] = _build_program()
    return _CACHE["nc"]


def _ensure_axon_hooks_importable():
    """bass_utils' trace branch imports antenv.axon_hooks, which the
    trimmed agent image may lack; stub it (hook=None) so a stray
    BASS_TRACE=1 degrades to an untraced run instead of crashing."""
    import sys
    import types

    try:
        import antenv.axon_hooks  # noqa: F401
        return
    except ImportError:
        pass
    mod = types.ModuleType("antenv.axon_hooks")
    mod._hook = None
    mod.get_axon_ntff_profile_hook = lambda: mod._hook

    def _set(h):
        mod._hook = h

    mod.set_axon_ntff_profile_hook = _set
    sys.modules["antenv.axon_hooks"] = mod
    try:
        import antenv
        antenv.axon_hooks = mod
    except ImportError:
        pass


def kernel(**inputs):
    global LAST_RESULTS
    from concourse.bass_utils import run_bass_kernel_spmd

    _ensure_axon_hooks_importable()

    x = np.ascontiguousarray(inputs["x"], dtype=np.float32)
    attn = np.ascontiguousarray(inputs["softmax_attention"], dtype=np.float32)
    w = np.ascontiguousarray(inputs["weight"], dtype=np.float32)
    b = np.ascontiguousarray(inputs["bias"], dtype=np.float32)

    nc = _get_program()
    in_maps = []
    for c in range(8):
        gb, go = divmod(c, _GRID_O)
        in_maps.append({
            "xT": np.ascontiguousarray(x[gb * _BL:(gb + 1) * _BL].T),
            "attn": np.ascontiguousarray(attn[gb * _BL:(gb + 1) * _BL]),
            "wT": np.ascontiguousarray(
                w[:, go * _OL:(go + 1) * _OL, :].transpose(0, 2, 1)
            ),
            "bias": np.ascontiguousarray(b[:, go * _OL:(go + 1) * _OL]),
        })

    res = run_bass_kernel_spmd(nc, in_maps, list(range(8)))
    LAST_RESULTS = res

    full = np.empty((_B, _OUT), dtype=np.float32)
    for c in range(8):
        gb, go = divmod(c, _GRID_O)
        full[gb * _BL:(gb + 1) * _BL, go * _OL:(go + 1) * _OL] = \
            res.results[c]["out"]
    return full
